# revision 1
# baseline (speedup 1.0000x reference)
"""Multi-head causal self-attention (B=4, T=2048, C=1024, 16 heads) on 8 trn2 cores.

Sharding: data-parallel over batch (4) x tensor-parallel over heads (2 groups of 8).
Core m handles batch m//2, head group m%2.

v2 design (vs baseline):
- No staging DMAs: DVE copies PSUM results straight into bf16 SBUF tiles
  (qT/kT/v/avT); only x/W loads, softmax-denominator broadcasts, the par1
  avT partition shift, and the output leave via DMA (the DMA device is a
  serialized resource in the cost model).
- Scores+AV matmuls run in bf16 (full PE rate at any moving width, so
  diagonal tiles cost exactly their width); projections stay f32r.
- Softmax denominators: ones column in v -> row 64 of the AV PSUM; DVE
  reciprocal + stride-0 DMA broadcast + DVE multiply (no ACT Ln/Exp, no
  PE ones-matmul).
- Phase interleave: QKV(s+1) / outproj(s-1) accumulation quanta are woven
  between attention chunk-pairs with a debt counter so the PE never idles
  on the exp (ACT) latency; ACT is ~78% of PE work total.
- Startup: x strip 0 and wq stream in per-c-chunk so the first matmul
  starts at ~2.5us instead of ~14.6us.
"""

import numpy as np

import concourse.bass as bass
import concourse.mybir as mybir
import concourse.tile as tile
from concourse.bass_utils import run_bass_kernel_spmd

F32 = mybir.dt.float32
F32R = mybir.dt.float32r
FP8 = mybir.dt.float8e4
PM = mybir.MatmulPerfMode
BF16 = mybir.dt.bfloat16
AF = mybir.ActivationFunctionType
MULT = mybir.AluOpType.mult
ADD = mybir.AluOpType.add

B, T, C = 4, 2048, 1024
HEADS, D = 16, 64
GROUPS = 2                  # head groups (tensor parallel)
HPC = HEADS // GROUPS       # heads per core = 8
GC = HPC * D                # group channel width = 512
NKC = T // 128              # Tk chunks = 16
NJ = T // 512               # Tq tiles = 4
CCH = C // 128              # contraction chunks = 8
NSTRIP = T // 512           # t strips = 4

_PROGRAM = None

# cost-model constants for the weave debt counter (ns)
PE_ROW = 0.4167             # ns per moving-dim row (full rate)
ACT_EL = 0.8333             # ns per free element on ACT
ACT_OH = 185.0              # fixed busy overhead per ACT instruction
QUANTUM = 4 * 512 * PE_ROW  # one dense quantum = 4 matmuls of N=512


def _patch_drain_chunking():
    """The axon walrus build rejects instructions with >~4 sem waits; Tile's
    kernel-tail drain waits on every live semaphore at once. Split it into a
    chain of drains with a bounded number of waits each."""
    from bass_rust import VectorClock, ScopedClock

    if getattr(tile.TileContext, "_drain_chunk_patched", False):
        return

    def _drain_and_barrier(self, tick_clock, wait_clock):
        gc_vec = list(tick_clock.global_clock)
        nz = [i for i, t in enumerate(gc_vec) if t > 0]
        CHUNK = 4
        for k in range(0, len(nz), CHUNK):
            keep = set(nz[k:k + CHUNK])
            partial = [gc_vec[i] if i in keep else 0 for i in range(len(gc_vec))]
            d = self.nc.sync.drain()
            wait_clock.add_sem_waits(d.ins, ScopedClock({None: VectorClock(partial)}))
        self.nc.all_engine_barrier()
        assert self.sems is not None
        popped = self.nc._tile_sem_poison_stack.pop()
        assert popped is self._sem_poison
        self.nc.clear_and_free_semaphores(list(self.sems.allocated().values()))
        self.nc.all_engine_barrier()

    tile.TileContext._drain_and_barrier = _drain_and_barrier
    tile.TileContext._drain_chunk_patched = True


def _split_excess_waits(nc, maxw=1, maxw_other=None):
    """Walrus rejects instructions carrying more than ~1 sem wait. Move excess
    waits onto same-engine NoOps inserted immediately before the instruction
    (engine streams execute in bb order, so semantics are preserved)."""
    from bass_rust import InstNoOp

    ctr = 0
    for f in nc.m.functions:
        for bb in f.blocks:
            new_insts = []
            for inst in bb.instructions:
                si = inst.sync_info
                waits = list(si.on_wait) if si and si.on_wait else []
                lim = maxw
                if maxw_other is not None and str(inst.engine) != 'EngineType.PE':
                    lim = maxw_other
                maxw_eff = lim
                if len(waits) > maxw_eff:
                    head, rest = waits[:-maxw_eff], waits[-maxw_eff:]
                    for k in range(0, len(head), maxw_eff):
                        ctr += 1
                        new_insts.append(InstNoOp(
                            name=f"waitnop_{ctr}",
                            engine=inst.engine,
                            sync_info=mybir.SyncInfo(
                                on_wait=head[k:k + maxw_eff], on_update=[]),
                        ))
                    inst.sync_info = mybir.SyncInfo(on_wait=rest, on_update=si.on_update)
                new_insts.append(inst)
            bb.instructions = new_insts
    return ctr


def _build_program():
    _patch_drain_chunking()
    nc = bass.Bass()

    xT_d = nc.declare_dram_parameter("xT", [C, T], BF16, isOutput=False)
    x8_d = nc.declare_dram_parameter("xT8", [C, T], FP8, isOutput=False)
    wq_d = nc.declare_dram_parameter("wqT", [C, GC], FP8, isOutput=False)
    wk_d = nc.declare_dram_parameter("wkT", [C, GC], FP8, isOutput=False)
    wv_d = nc.declare_dram_parameter("wvT", [C, GC], BF16, isOutput=False)
    wp_d = nc.declare_dram_parameter("wpT", [GC, C], BF16, isOutput=False)
    out_d = nc.declare_dram_parameter("outp", [T, C], F32, isOutput=True)

    from contextlib import ExitStack

    with tile.TileContext(nc) as tc, ExitStack() as stack:
        ep = stack.enter_context
        persist = ep(tc.tile_pool(name="persist", bufs=1))
        xs_pool = ep(tc.tile_pool(name="xs", bufs=2))
        pt_pool = ep(tc.tile_pool(name="pt", bufs=8))
        t8_pool = ep(tc.tile_pool(name="t8", bufs=3))
        avr_pool = ep(tc.tile_pool(name="avr", bufs=3))
        r_pool = ep(tc.tile_pool(name="rr", bufs=3))
        rb_pool = ep(tc.tile_pool(name="rb", bufs=3))
        avf_pool = ep(tc.tile_pool(name="avf", bufs=2))
        ob_pool = ep(tc.tile_pool(name="ob", bufs=2))
        dense_ps = ep(tc.tile_pool(name="dps", bufs=2, space="PSUM"))
        ps_s = ep(tc.tile_pool(name="pss", bufs=2, space="PSUM"))
        ps_av = ep(tc.tile_pool(name="psav", bufs=1, space="PSUM"))

        # per-strip q/k tiles, bf16 (scores operands stay bf16 for precision;
        # per-strip tiles because dep tracking is per-tile and one big tile
        # would serialize the weave)
        qTs = [persist.tile([128, HPC // 2, 512], BF16, name=f"qT_{s}")
               for s in range(NSTRIP)]
        kTs = [persist.tile([128, HPC // 2, 512], BF16, name=f"kT_{s}")
               for s in range(NSTRIP)]
        # v padded with a ones column per head: [t-chunk-part, chunk, head, 65]
        vs = [persist.tile([128, 4, HPC, D + 1], BF16, name=f"v_{s}")
              for s in range(NSTRIP)]
        for vt in vs:
            # memset first: ACT scale=0.0 on uninit garbage yields NaN*0=NaN on HW
            nc.gpsimd.memset(vt[:, :, :, D:D + 1], 0.0)
            nc.scalar.activation(vt[:, :, :, D:D + 1], vt[:, :, :, D:D + 1],
                                 AF.Copy, scale=0.0, bias=1.0)
        # avT split per (strip, head-pair): dep tracking is per-tile, so any
        # coarser layout makes outproj readers serialize behind the LATEST
        # writer of the shared tile (e.g. hp3's normalize), not their own
        avTs = [[persist.tile([128, 512], BF16, name=f"avT{s}_{h}")
                 for h in range(HPC // 2)] for s in range(NSTRIP)]

        # fp8 DoubleRow packing for q/k projections: input channel d=2*lane+i,
        # so a [256-row, N] DRAM block DMAs into [128, 2, N] in natural order
        wq = persist.tile([128, C // 256, 2, GC], FP8)
        wk = persist.tile([128, C // 256, 2, GC], FP8)
        wv = persist.tile([128, CCH, GC], BF16)
        wp = persist.tile([128, GC // 128, C], BF16)

        # ---- startup DMAs: x strip 0 + wq per c-chunk (interleaved), then wk/wv ----
        xs_tiles = {}
        x8_tiles = {}

        def load_strip8(s):
            t8x = xs_pool.tile([128, C // 256, 2, 512], FP8, tag="xs8", name=f"x8_{s}")
            x8_tiles[s] = t8x
            for c2 in range(C // 256):
                nc.sync.dma_start(
                    t8x[:, c2, :, :],
                    x8_d[256 * c2:256 * (c2 + 1), 512 * s:512 * (s + 1)]
                    .rearrange("(c p) t -> p c t", p=256))

        load_strip8(0)
        for c2 in range(C // 256):
            nc.sync.dma_start(
                wq[:, c2, :, :],
                wq_d[256 * c2:256 * (c2 + 1), :].rearrange("(c p) o -> p c o", p=256))
        for c2 in range(C // 256):
            nc.sync.dma_start(
                wk[:, c2, :, :],
                wk_d[256 * c2:256 * (c2 + 1), :].rearrange("(c p) o -> p c o", p=256))
        xs_tiles[0] = xs_pool.tile([128, CCH, 512], BF16, tag="xs", name="xs0")
        for c in range(CCH):
            nc.sync.dma_start(
                xs_tiles[0][:, c, :],
                xT_d[128 * c:128 * (c + 1), 0:512].rearrange("(c p) t -> p c t", p=128))

        # PE p-state warm-up: the tensor engine runs 2-3.7x slower until it has
        # been continuously busy ~3us. Burn the startup DMA wait on dummy
        # matmuls so the real projections start at full clock.
        warm = persist.tile([128, 128], BF16, name="warm")
        nc.gpsimd.memset(warm[:, :], 0.0)
        wps = dense_ps.tile([128, 128], F32, tag="dp", name="wps")
        NWARM = 12
        for w in range(NWARM):
            nc.tensor.matmul(wps[:, :], warm[:, :], warm[:, :],
                             start=(w == 0), stop=(w == NWARM - 1))
        def load_wv_wp():
            for c in range(CCH):
                nc.sync.dma_start(wv[:, c, :],
                                  wv_d[128 * c:128 * (c + 1), :]
                                  .rearrange("(c p) o -> p c o", p=128))
            for c in range(GC // 128):
                nc.sync.dma_start(wp[:, c, :],
                                  wp_d[128 * c:128 * (c + 1), :]
                                  .rearrange("(c p) o -> p c o", p=128))

        def load_strip(s):
            """Emit chunked DMA loads for x strip s (s>0)."""
            load_strip8(s)
            t = xs_pool.tile([128, CCH, 512], BF16, tag="xs", name=f"xs{s}")
            xs_tiles[s] = t
            for c in range(CCH):
                nc.sync.dma_start(
                    t[:, c, :],
                    xT_d[128 * c:128 * (c + 1), 512 * s:512 * (s + 1)]
                    .rearrange("(c p) t -> p c t", p=128))

        # ---- dense work generators (each yield = ~one QUANTUM of PE work) ----
        def gen_qk(s, o, w_sb, dsts):
            """q or k projection group: out channels [128o:128o+128] of strip s.
            Output is staged to fp8 and DMA'd into the DoubleRow-packed layout
            (d=2*lane+half interleave makes it a natural-order copy)."""
            x8 = x8_tiles[s]
            dst = dsts[s]
            pq = dense_ps.tile([128, 512], F32, tag="dp", name="pq")
            for c2 in range(C // 256):
                nc.tensor.matmul(pq[:, :], w_sb[:, c2, :, 128 * o:128 * (o + 1)],
                                 x8[:, c2, :, :], start=(c2 == 0),
                                 stop=(c2 == C // 256 - 1), perf_mode=PM.DoubleRow)
            if s == 0 or flush_mode[0]:
                # flush bursts run at ATT boundaries where ACT is briefly idle;
                # DVE there is backlogged with normalize chains and would hold
                # the dense bank
                nc.scalar.activation(dst[:, o, :], pq[:, :], AF.Copy)
            else:
                nc.vector.tensor_copy(dst[:, o, :], pq[:, :])
            yield 427.0

        def gen_v(s, tt):
            """v projection group: t chunk 4s+tt, all 8 heads."""
            xs = xs_tiles[s]
            pv = dense_ps.tile([128, 512], F32, tag="dp", name="pv")
            for c in range(4):
                nc.tensor.matmul(pv[:, :], xs[:, c, 128 * tt:128 * (tt + 1)],
                                 wv[:, c, :], start=(c == 0), stop=False)
            yield
            for c in range(4, CCH):
                nc.tensor.matmul(pv[:, :], xs[:, c, 128 * tt:128 * (tt + 1)],
                                 wv[:, c, :], start=False, stop=(c == CCH - 1))
            if s == 0:
                nc.scalar.activation(vs[s][:, tt, :, 0:D],
                                     pv[:, :].rearrange("p (h d) -> p h d", h=HPC), AF.Copy)
            else:
                nc.vector.tensor_copy(
                    vs[s][:, tt, :, 0:D],
                    pv[:, :].rearrange("p (h d) -> p h d", h=HPC))
            yield

        deferred_dmas = []

        def gen_po(tt, pool=None, cp="dve", defer_dma=False):
            """output projection for t chunk tt (both 512-col halves)."""
            pool = pool or dense_ps
            tail = pool is not dense_ps
            ob = ob_pool.tile([128, C], F32, tag="ob", name="ob")
            for o2 in range(2):
                po = pool.tile([128, 512], F32, tag="dp" if not tail else "s",
                               name="po")
                tl = tt % 4
                for c4 in range(GC // 128):
                    nc.tensor.matmul(po[:, :],
                                     avTs[tt // 4][c4][:, 128 * tl:128 * (tl + 1)],
                                     wp[:, c4, 512 * o2:512 * (o2 + 1)],
                                     start=(c4 == 0), stop=(c4 == GC // 128 - 1))
                if tail:
                    if o2 == 0:
                        nc.scalar.activation(ob[:, 512 * o2:512 * (o2 + 1)], po[:, :], AF.Copy)
                    else:
                        nc.vector.tensor_copy(ob[:, 512 * o2:512 * (o2 + 1)], po[:, :])
                    nc.sync.dma_start(
                        out_d[128 * tt:128 * (tt + 1), 512 * o2:512 * (o2 + 1)],
                        ob[:, 512 * o2:512 * (o2 + 1)])
                elif cp == "act":
                    nc.scalar.activation(ob[:, 512 * o2:512 * (o2 + 1)], po[:, :], AF.Copy)
                else:
                    nc.vector.tensor_copy(ob[:, 512 * o2:512 * (o2 + 1)], po[:, :])
                yield
            if not tail:
                if defer_dma:
                    deferred_dmas.append((tt, ob))
                else:
                    for o2 in range(2):
                        nc.sync.dma_start(
                            out_d[128 * tt:128 * (tt + 1), 512 * o2:512 * (o2 + 1)],
                            ob[:, 512 * o2:512 * (o2 + 1)])

        # ---- weave machinery ----
        flush_mode = [False]
        queue = []          # pending dense generators (FIFO)
        debt = [0.0]

        def weave(extra_ns):
            debt[0] += extra_ns
            while debt[0] >= QUANTUM * 0.5 and queue:
                g = queue[0]
                try:
                    cost = next(g)
                    debt[0] -= cost if cost else QUANTUM
                except StopIteration:
                    queue.pop(0)

        def flush():
            flush_mode[0] = True
            while queue:
                g = queue.pop(0)
                for _ in g:
                    pass
            debt[0] = 0.0
            flush_mode[0] = False

        def enqueue_qkv(s):
            for o in range(HPC // 2):
                queue.append(gen_qk(s, o, wq, qTs))
            for o in range(HPC // 2):
                queue.append(gen_qk(s, o, wk, kTs))
            for tt in range(4):
                queue.append(gen_v(s, tt))

        # ---- attention ----
        def emit_att(j, hp):
            nkc = 4 * (j + 1)
            # both pars live in one 2-bank PSUM tile (cols [0:512] / [512:1024])
            av = ps_av.tile([65, 1024], F32, tag="av", name="av")

            def s_pair(i):
                roff = max(0, 128 * i - 512 * j)
                diag = 128 * i - 512 * j >= 0
                sps = ps_s.tile([128, 1024], F32, tag="s", name="sps")
                for par in range(2):
                    pb = 64 * par
                    nc.tensor.matmul(
                        sps[:, 512 * par + roff:512 * (par + 1)],
                        kTs[i // 4][pb:pb + 64, hp, 128 * (i % 4):128 * (i % 4 + 1)],
                        qTs[j][pb:pb + 64, hp, roff:512],
                        start=True, stop=True)
                ptile = pt_pool.tile([128, 1024], BF16, tag="pt", name="pt")
                # one exp for both pars: strided AP over the two [roff:512] ranges
                nc.scalar.activation(
                    ptile.rearrange("p (a b) -> p a b", a=2)[:, :, roff:512],
                    sps.rearrange("p (a b) -> p a b", a=2)[:, :, roff:512],
                    AF.Exp, scale=0.125)
                if diag:
                    for par in range(2):
                        # causal mask: zero p above the diagonal (Pool, SBUF-only)
                        nc.gpsimd.affine_select(
                            out=ptile[:, 512 * par + roff:512 * par + roff + 128],
                            in_=ptile[:, 512 * par + roff:512 * par + roff + 128],
                            compare_op=mybir.AluOpType.is_ge, fill=0.0, base=0,
                            pattern=[[1, 128]], channel_multiplier=-1,
                        )
                return (ptile, roff)

            def av_pair(i, pts):
                ptile, roff = pts
                for par in range(2):
                    nc.tensor.matmul(
                        av[:, 512 * par + roff:512 * (par + 1)],
                        vs[i // 4][:, i % 4, 2 * hp + par, :],
                        ptile[:, 512 * par + roff:512 * (par + 1)],
                        start=(i == 0), stop=(i == nkc - 1))

            def deficit(i):
                roff = max(0, 128 * i - 512 * j)
                n = 512 - roff
                act = ACT_OH + ACT_EL * 2 * n
                pe = 4 * n * PE_ROW
                return act - pe

            prev = s_pair(0)
            for i in range(1, nkc):
                cur = s_pair(i)
                weave(deficit(i - 1))
                av_pair(i - 1, prev)
                prev = cur
            weave(deficit(nkc - 1))
            av_pair(nkc - 1, prev)

            # normalize: denom at row 64 of av PSUM (ones column of v).
            # One copy/reciprocal/broadcast covers both pars; par1's multiply
            # goes first (it has the extra partition-shift DMA on its path).
            # Multiplies run on Pool (all-SBUF) so the DVE stream never gates
            # the dense-bank copies; the last strip stages via ACT to keep
            # DVE out of the tail chain entirely.
            last = (j == NJ - 1 and hp == HPC // 2 - 1)
            if last:
                # tail path: DVE is idle here — skip staging, read av straight
                # from PSUM, per-par chains with par1 (the DMA-shifted one) first
                r = r_pool.tile([65, 1024], F32, tag="r", name="r")
                rb = rb_pool.tile([64, 1024], F32, tag="rb", name="rb")
                for par in (1, 0):
                    cs = slice(512 * par, 512 * (par + 1))
                    nc.vector.reciprocal(r[64:65, cs], av[64:65, cs])
                    nc.sync.dma_start(
                        rb[0:64, cs],
                        r[64:65, cs].unsqueeze(1).broadcast_to([1, 64, 512]))
                avf = avf_pool.tile([64, 512], BF16, tag="avf", name="avf")
                nc.vector.tensor_tensor(avf[:, :], av[0:64, 512:1024],
                                        rb[0:64, 512:1024], op=MULT)
                nc.sync.dma_start(avTs[j][hp][64:128, :], avf[:, :])
                nc.vector.tensor_tensor(
                    avTs[j][hp][0:64, :],
                    av[0:64, 0:512], rb[0:64, 0:512], op=MULT)
            else:
                asrc = avr_pool.tile([65, 1024], F32, tag="avr", name="avr")
                nc.vector.tensor_copy(asrc[:, :], av[:, :])
                r = r_pool.tile([65, 1024], F32, tag="r", name="r")
                nc.vector.reciprocal(r[64:65, :], asrc[64:65, :])
                rb = rb_pool.tile([64, 1024], F32, tag="rb", name="rb")
                nc.sync.dma_start(
                    rb[0:64, :], r[64:65, :].unsqueeze(1).broadcast_to([1, 64, 1024]))
                avf = avf_pool.tile([64, 512], BF16, tag="avf", name="avf")
                nc.gpsimd.tensor_tensor(avf[:, :], asrc[0:64, 512:1024],
                                        rb[0:64, 512:1024], op=MULT)
                nc.sync.dma_start(avTs[j][hp][64:128, :], avf[:, :])
                nc.gpsimd.tensor_tensor(
                    avTs[j][hp][0:64, :],
                    asrc[0:64, 0:512], rb[0:64, 0:512], op=MULT)

        # ---- main schedule ----
        # QKV(0): interleave pairs of groups so the first group doesn't
        # solo-wait on all 8 chunk DMAs
        # q/k groups first so their packing DMAs reach the device before the
        # wv/wp streams (ATT(0) waits on them); wv/wp issue just before v groups
        def run_pairs(gens):
            for a, b in zip(gens[0::2], gens[1::2]):
                for g in (a, b, a, b):
                    try:
                        next(g)
                    except StopIteration:
                        pass
        run_pairs([gen_qk(0, o, wq, qTs) for o in range(HPC // 2)]
                  + [gen_qk(0, o, wk, kTs) for o in range(HPC // 2)])
        load_wv_wp()
        run_pairs([gen_v(0, tt) for tt in range(4)])
        for j in range(NJ):
            if j + 1 < NSTRIP:
                load_strip(j + 1)      # x strip j+1 streams during ATT(j)
                enqueue_qkv(j + 1)     # woven into ATT(j)
            reserve = []
            if j == NJ - 1:
                for tt in range(0, 4 * j):
                    queue.append(gen_po(tt))   # PO(0..2) woven into ATT(3)
            for hp in range(HPC // 2):
                emit_att(j, hp)
            for g in reserve:          # fills the last hp's normalize latency
                for _ in g:
                    pass
            for tt, ob in deferred_dmas:   # now queue behind the hp3 rb/avf DMAs
                for o2 in range(2):
                    nc.sync.dma_start(
                        out_d[128 * tt:128 * (tt + 1), 512 * o2:512 * (o2 + 1)],
                        ob[:, 512 * o2:512 * (o2 + 1)])
            deferred_dmas.clear()
            if j + 1 < NSTRIP:
                flush()                # QKV(j+1) must be fully emitted before ATT(j+1)
        flush()
        # tail outproj in waves of 4 concurrent PSUM groups: all c4=0..2
        # matmuls are emitted before the first c4=3, so the in-order PE
        # stream has ~2.6us of real work to run while the last head-pair's
        # normalize chain (which c4=3 needs) completes
        for wave in range(1):
            gps = []
            obs = {}
            for tt in (4 * (NJ - 1) + 2 * wave, 4 * (NJ - 1) + 2 * wave + 1):
                obs[tt] = ob_pool.tile([128, C], F32, tag="ob", name="ob")
                for o2 in range(2):
                    pool = ps_s if len(gps) % 2 == 0 else dense_ps
                    po = pool.tile([128, 512], F32,
                                   tag="s" if pool is ps_s else "dp", name="po")
                    gps.append((tt, o2, po))
            for c4 in range(GC // 128 - 1):
                for tt, o2, po in gps:
                    tl = tt % 4
                    nc.tensor.matmul(po[:, :],
                                     avTs[NJ - 1][c4][:, 128 * tl:128 * (tl + 1)],
                                     wp[:, c4, 512 * o2:512 * (o2 + 1)],
                                     start=(c4 == 0), stop=False)
            # zero-contribution matmuls (lhsT is the zeroed warm tile) bridge
            # the residual normalize wait so the finishers run at full clock
            for _ in range(2):
                for tt, o2, po in gps:
                    nc.tensor.matmul(po[:, :], warm[:, :], qTs[0][:, 0, :],
                                     start=False, stop=False)
            for gi, (tt, o2, po) in enumerate(gps):
                tl = tt % 4
                nc.tensor.matmul(po[:, :],
                                 avTs[NJ - 1][3][:, 128 * tl:128 * (tl + 1)],
                                 wp[:, 3, 512 * o2:512 * (o2 + 1)],
                                 start=False, stop=True)
                if gi % 2 == 0:
                    nc.scalar.activation(obs[tt][:, 512 * o2:512 * (o2 + 1)],
                                         po[:, :], AF.Copy)
                else:
                    nc.vector.tensor_copy(obs[tt][:, 512 * o2:512 * (o2 + 1)],
                                          po[:, :])
                nc.sync.dma_start(
                    out_d[128 * tt:128 * (tt + 1), 512 * o2:512 * (o2 + 1)],
                    obs[tt][:, 512 * o2:512 * (o2 + 1)])
        for tt in range(4 * (NJ - 1) + 2, 4 * NJ):
            g = gen_po(tt, pool=ps_s)
            for _ in g:
                pass
    _split_excess_waits(nc)
    return nc


def _get_program():
    global _PROGRAM
    if _PROGRAM is None:
        _PROGRAM = _build_program()
    return _PROGRAM


def _make_in_maps(x, Wk, Wq, Wv, Wp):
    import ml_dtypes
    x = np.asarray(x, dtype=np.float32)
    Wk = np.asarray(Wk, dtype=np.float32)
    Wq = np.asarray(Wq, dtype=np.float32)
    Wv = np.asarray(Wv, dtype=np.float32)
    Wp = np.asarray(Wp, dtype=np.float32)
    in_maps = []
    for core in range(8):
        b, g = core // GROUPS, core % GROUPS
        rows = slice(GC * g, GC * (g + 1))
        in_maps.append({
            "xT": np.ascontiguousarray(x[b].T).astype(ml_dtypes.bfloat16),
            "xT8": np.ascontiguousarray(x[b].T).astype(ml_dtypes.float8_e4m3),
            "wqT": np.ascontiguousarray(Wq[rows, :].T).astype(ml_dtypes.float8_e4m3),
            "wkT": np.ascontiguousarray(Wk[rows, :].T).astype(ml_dtypes.float8_e4m3),
            "wvT": np.ascontiguousarray(Wv[rows, :].T).astype(ml_dtypes.bfloat16),
            "wpT": np.ascontiguousarray(Wp[:, rows].T).astype(ml_dtypes.bfloat16),
        })
    return in_maps


def run(x, Wk, Wq, Wv, Wp, bp, trace=False, **spmd_kwargs):
    nc = _get_program()
    in_maps = _make_in_maps(x, Wk, Wq, Wv, Wp)
    res = run_bass_kernel_spmd(nc, in_maps, list(range(8)), trace=trace, **spmd_kwargs)
    bp = np.asarray(bp, dtype=np.float32)
    out = np.empty((B, T, C), dtype=np.float32)
    for b in range(B):
        out[b] = res.results[GROUPS * b]["outp"] + res.results[GROUPS * b + 1]["outp"] + bp
    return out, res


def kernel(x, Wk, Wq, Wv, Wp, bp):
    out, _ = run(x, Wk, Wq, Wv, Wp, bp)
    return out



# revision 57
# speedup vs baseline: 1.0921x; 1.0921x over previous
"""Multi-head causal self-attention (B=4, T=2048, C=1024, 16 heads) on 8 trn2 cores.

Sharding: data-parallel over batch (4) x tensor-parallel over heads (2 groups of 8).
Core m handles batch m//2, head group m%2.

v3 design (vs v2 baseline at ~202us):
- Scores matmuls run fp8e4 + DoubleRow (0.5 cyc/row): q/k are staged to fp8
  right out of the projection PSUM and repacked into [32-lane, 2-interleave]
  DoubleRow layout by two plain partition-block DMAs per strip (the host
  permutes Wq/Wk columns so channel 2l+i lands at partition 64i+32par+l,
  making the repack a contiguous partition-block copy). Scores PE time
  halves: 58us -> 29us.
- AV matmuls are flipped: stationary = p chunk [128k x 128q], moving =
  v [128k x 65(d+ones)], out = [q-part, 65]. Output free size drops from
  512-wide q to 65-wide d: AV PE time 58us -> ~30us. The denominator
  arrives per-q-partition (ones column of v), so the softmax normalize is a
  per-partition reciprocal + free-dim stride-0 broadcast multiply -- no
  cross-partition DMA broadcast, no partition-shift DMA.
- avT (needed [d, t] for the output projection) comes from PE transposes of
  the normalized [q, d] tile via an identity matmul (f32, 2cyc/row,
  ~107ns/tile), written into a dense-pool PSUM bank and copied once.
- The normalize+transpose finalize of (j,hp) is deferred into the next
  (j,hp)'s chunk stream so the PE doesn't stall on the DVE/Pool latency.
- Weave debt counter refit for the new per-chunk ACT/PE balance.
"""

import numpy as np

import concourse.bass as bass
import concourse.mybir as mybir
import concourse.tile as tile
from concourse.bass_utils import run_bass_kernel_spmd

F32 = mybir.dt.float32
F32R = mybir.dt.float32r
FP8 = mybir.dt.float8e4
PM = mybir.MatmulPerfMode
BF16 = mybir.dt.bfloat16
AF = mybir.ActivationFunctionType
MULT = mybir.AluOpType.mult
ADD = mybir.AluOpType.add

B, T, C = 4, 2048, 1024
HEADS, D = 16, 64
GROUPS = 2                  # head groups (tensor parallel)
HPC = HEADS // GROUPS       # heads per core = 8
GC = HPC * D                # group channel width = 512
NKC = T // 128              # Tk chunks = 16
NJ = T // 512               # Tq tiles = 4
CCH = C // 128              # contraction chunks = 8
NSTRIP = T // 512           # t strips = 4

_PROGRAM = None

# cost-model constants for the weave debt counter (ns)
PE_ROW = 0.4167             # ns per moving-dim row (full rate)
ACT_EL = 0.8333             # ns per free element on ACT
ACT_OH = 185.0              # fixed busy overhead per ACT instruction
QUANTUM = 4 * 512 * PE_ROW  # one dense quantum = 4 matmuls of N=512


def _patch_drain_chunking():
    """The axon walrus build rejects instructions with >~4 sem waits; Tile's
    kernel-tail drain waits on every live semaphore at once. Split it into a
    chain of drains with a bounded number of waits each."""
    from bass_rust import VectorClock, ScopedClock

    if getattr(tile.TileContext, "_drain_chunk_patched", False):
        return

    def _drain_and_barrier(self, tick_clock, wait_clock):
        gc_vec = list(tick_clock.global_clock)
        nz = [i for i, t in enumerate(gc_vec) if t > 0]
        CHUNK = 4
        for k in range(0, len(nz), CHUNK):
            keep = set(nz[k:k + CHUNK])
            partial = [gc_vec[i] if i in keep else 0 for i in range(len(gc_vec))]
            d = self.nc.sync.drain()
            wait_clock.add_sem_waits(d.ins, ScopedClock({None: VectorClock(partial)}))
        self.nc.all_engine_barrier()
        assert self.sems is not None
        popped = self.nc._tile_sem_poison_stack.pop()
        assert popped is self._sem_poison
        self.nc.clear_and_free_semaphores(list(self.sems.allocated().values()))
        self.nc.all_engine_barrier()

    tile.TileContext._drain_and_barrier = _drain_and_barrier
    tile.TileContext._drain_chunk_patched = True


def _split_excess_waits(nc, maxw=1, maxw_other=None):
    """Walrus rejects instructions carrying more than ~1 sem wait. Move excess
    waits onto same-engine NoOps inserted immediately before the instruction
    (engine streams execute in bb order, so semantics are preserved)."""
    from bass_rust import InstNoOp

    ctr = 0
    for f in nc.m.functions:
        for bb in f.blocks:
            new_insts = []
            for inst in bb.instructions:
                si = inst.sync_info
                waits = list(si.on_wait) if si and si.on_wait else []
                lim = maxw
                if maxw_other is not None and str(inst.engine) != 'EngineType.PE':
                    lim = maxw_other
                maxw_eff = lim
                if len(waits) > maxw_eff:
                    head, rest = waits[:-maxw_eff], waits[-maxw_eff:]
                    for k in range(0, len(head), maxw_eff):
                        ctr += 1
                        new_insts.append(InstNoOp(
                            name=f"waitnop_{ctr}",
                            engine=inst.engine,
                            sync_info=mybir.SyncInfo(
                                on_wait=head[k:k + maxw_eff], on_update=[]),
                        ))
                    inst.sync_info = mybir.SyncInfo(on_wait=rest, on_update=si.on_update)
                new_insts.append(inst)
            bb.instructions = new_insts
    return ctr


def _build_program():
    _patch_drain_chunking()
    nc = bass.Bass()

    xT_d = nc.declare_dram_parameter("xT", [C, T], BF16, isOutput=False)
    x8_d = nc.declare_dram_parameter("xT8", [C, T], FP8, isOutput=False)
    wq_d = nc.declare_dram_parameter("wqT", [C, GC], FP8, isOutput=False)
    wk_d = nc.declare_dram_parameter("wkT", [C, GC], FP8, isOutput=False)
    wv_d = nc.declare_dram_parameter("wvT", [C, GC], BF16, isOutput=False)
    wp_d = nc.declare_dram_parameter("wpT", [GC, C], BF16, isOutput=False)
    id_d = nc.declare_dram_parameter("ident", [128, 128], F32, isOutput=False)
    out_d = nc.declare_dram_parameter("outp", [T, C], BF16, isOutput=True)

    from contextlib import ExitStack

    with tile.TileContext(nc) as tc, ExitStack() as stack:
        ep = stack.enter_context
        persist = ep(tc.tile_pool(name="persist", bufs=1))
        xs_pool = ep(tc.tile_pool(name="xs", bufs=2))
        st_pool = ep(tc.tile_pool(name="st8", bufs=3))
        pt_pool = ep(tc.tile_pool(name="pt", bufs=30))
        avn_pool = ep(tc.tile_pool(name="avn", bufs=2))
        r_pool = ep(tc.tile_pool(name="rr", bufs=2))
        ob_pool = ep(tc.tile_pool(name="ob", bufs=2))
        dense_ps = ep(tc.tile_pool(name="dps", bufs=2, space="PSUM"))
        ps_s = ep(tc.tile_pool(name="pss", bufs=2, space="PSUM"))
        ps_av = ep(tc.tile_pool(name="psav", bufs=1, space="PSUM"))

        # DoubleRow-packed q/k per strip: [lane(32par+l), o-group, i, t]
        # channel d = 2l+i of head (o, par); host perm makes the repack a
        # plain partition-block DMA (i half -> lane block).
        qPs = [persist.tile([64, 4, 2, 512], FP8, name=f"qP_{s}")
               for s in range(NSTRIP)]
        kPs = [persist.tile([64, 4, 2, 512], FP8, name=f"kP_{s}")
               for s in range(NSTRIP)]
        # v padded with a ones column per head: [t-chunk-part, chunk, head, 65]
        vs = [persist.tile([128, 4, HPC, D + 1], BF16, name=f"v_{s}")
              for s in range(NSTRIP)]
        for vt in vs:
            # ones column via DVE only (memset then +1) -- keeping this off
            # Pool/ACT keeps the in-order ACT exp stream free of init waits
            nc.vector.memset(vt[:, :, :, D:D + 1], 0.0)
            nc.vector.tensor_scalar_add(vt[:, :, :, D:D + 1],
                                        vt[:, :, :, D:D + 1], 1.0)
        # avT per (strip, head-pair): [part(64par+d), t] bf16 for the out-proj
        avTs = [[persist.tile([128, 512], BF16, name=f"avT{s}_{h}")
                 for h in range(HPC // 2)] for s in range(NSTRIP)]

        # fp8 DoubleRow packing for q/k projections: input channel d=2*lane+i,
        # so a [256-row, N] DRAM block DMAs into [128, 2, N] in natural order
        wq = persist.tile([128, C // 256, 2, GC], FP8)
        wk = persist.tile([128, C // 256, 2, GC], FP8)
        wv = persist.tile([128, CCH, GC], BF16)
        wp = persist.tile([128, GC // 128, C], BF16)
        ident = persist.tile([128, 128], F32, name="ident")

        # ---- startup DMAs: x strip 0 + wq per c-chunk (interleaved), then wk/wv ----
        xs_tiles = {}
        x8_tiles = {}

        def load_strip8(s):
            t8x = xs_pool.tile([128, C // 256, 2, 512], FP8, tag="xs8", name=f"x8_{s}")
            x8_tiles[s] = t8x
            for c2 in range(C // 256):
                nc.sync.dma_start(
                    t8x[:, c2, :, :],
                    x8_d[256 * c2:256 * (c2 + 1), 512 * s:512 * (s + 1)]
                    .rearrange("(c p) t -> p c t", p=256))

        # Priority startup loads: x strip-0 fp8 + the o-group-0 columns of
        # wq/wk only (6 DMAs), so the first q/k projection and ATT(0,0) start
        # ~4us in. The remaining weight columns and the bulky bf16 loads are
        # emitted after the inline o0 work (wv/xs0/wp via the Pool SWDGE
        # queue so they don't contend for HWDGE).
        t8x0 = xs_pool.tile([128, C // 256, 2, 512], FP8, tag="xs8", name="x8_0")
        x8_tiles[0] = t8x0
        for two in range(2):
            nc.sync.dma_start(
                t8x0[:, :, two, :],
                x8_d[:, 0:512]
                .rearrange("(c p two) t -> p c two t", p=128, two=2)[:, :, two, :])
        for w_sb, w_dram in ((wq, wq_d), (wk, wk_d)):
            for two in range(2):
                nc.sync.dma_start(
                    w_sb[:, :, two, 0:128],
                    w_dram[:, 0:128]
                    .rearrange("(c p two) o -> p c two o", p=128, two=2)[:, :, two, :])

        def load_qk_rest():
            for w_sb, w_dram in ((wq, wq_d), (wk, wk_d)):
                for two in range(2):
                    nc.scalar.dma_start(
                        w_sb[:, :, two, 128:GC],
                        w_dram[:, 128:GC]
                        .rearrange("(c p two) o -> p c two o", p=128, two=2)[:, :, two, :])

        def load_xs0_wv():
            # bulk loads on the Pool SWDGE queue, ordered by need-by time; the
            # ~1.2us per-dispatch SWDGE cost also keeps their transfers from
            # stealing the DMA device from the priority q/k-path loads
            xs_tiles[0] = xs_pool.tile([128, CCH, 512], BF16, tag="xs", name="xs0")
            nc.gpsimd.dma_start(
                xs_tiles[0][:, :, 0:256],
                xT_d[:, 0:256].rearrange("(c p) t -> p c t", p=128))
            nc.gpsimd.dma_start(wv[:, :, :],
                                wv_d[:, :].rearrange("(c p) o -> p c o", p=128))
            nc.gpsimd.dma_start(
                xs_tiles[0][:, :, 256:512],
                xT_d[:, 256:512].rearrange("(c p) t -> p c t", p=128))
            nc.gpsimd.dma_start(ident[:, :], id_d[:, :])

        # PE p-state warm-up: the tensor engine runs 2-3.7x slower until it has
        # been continuously busy ~3us. Burn the startup DMA wait on dummy
        # matmuls so the real projections start at full clock.
        warm = persist.tile([128, 128], BF16, name="warm")
        nc.gpsimd.memset(warm[:, :], 0.0)
        wps = dense_ps.tile([128, 128], F32, tag="dp", name="wps")
        NWARM = 12
        for w in range(NWARM):
            nc.tensor.matmul(wps[:, :], warm[:, :], warm[:, :],
                             start=(w == 0), stop=(w == NWARM - 1))

        def load_wv_wp():
            nc.gpsimd.dma_start(wv[:, :, :],
                                wv_d[:, :].rearrange("(c p) o -> p c o", p=128))
            nc.gpsimd.dma_start(wp[:, :, :],
                                wp_d[:, :].rearrange("(c p) o -> p c o", p=128))

        def load_strip(s):
            """Emit DMA loads for x strip s (s>0): one coarse bf16 load on the
            Pool SWDGE queue + two fp8 halves on the scalar queue (keeps the
            sync queue short for the latency-critical repacks)."""
            t8x = xs_pool.tile([128, C // 256, 2, 512], FP8, tag="xs8", name=f"x8_{s}")
            x8_tiles[s] = t8x
            for two in range(2):
                nc.gpsimd.dma_start(
                    t8x[:, :, two, :],
                    x8_d[:, 512 * s:512 * (s + 1)]
                    .rearrange("(c p two) t -> p c two t", p=128, two=2)[:, :, two, :])
            t = xs_pool.tile([128, CCH, 512], BF16, tag="xs", name=f"xs{s}")
            xs_tiles[s] = t
            for ch in range(0, CCH, 2):
                nc.sync.dma_start(
                    t[:, ch:ch + 2, :],
                    xT_d[128 * ch:128 * (ch + 2), 512 * s:512 * (s + 1)]
                    .rearrange("(c p) t -> p c t", p=128))

        # ---- markers / guards ----
        v_done = set()       # (strip, tt) whose v copy has been emitted
        rep_done = set()     # (tensor, strip, o) repack DMA emitted
        fin_done = set()     # (j, hp) finalize (avT write) emitted

        # ---- dense work generators (each yield = ~one QUANTUM of PE work) ----
        def gen_qk(s, o, w_sb, stage):
            """q or k projection group: out channels [128o:128o+128] of strip s
            (host-permuted order), staged to fp8."""
            x8 = x8_tiles[s]
            pq = dense_ps.tile([128, 512], F32, tag="dp", name="pq")
            for c2 in range(C // 256):
                nc.tensor.matmul(pq[:, :], w_sb[:, c2, :, 128 * o:128 * (o + 1)],
                                 x8[:, c2, :, :], start=(c2 == 0),
                                 stop=(c2 == C // 256 - 1), perf_mode=PM.DoubleRow)
            nc.vector.tensor_copy(stage[:, o, :], pq[:, :])
            yield 427.0

        def gen_repack(s, tensor, stage, dstP, olist=None):
            """Repack the fp8 stage [128(ch), 4, 512] into the DoubleRow tile
            [64(lane), 4, 2, 512]: channel at partition 64i+32par+l (host perm)
            goes to lane 32par+l, interleave slot i. olist=None repacks the
            whole strip in 2 DMAs; an explicit olist does per-o-group DMAs
            (used at startup so ATT(0,hp) can begin before all groups run)."""
            if olist is None:
                for i in range(2):
                    nc.sync.dma_start(dstP[0:64, :, i, :],
                                      stage[64 * i:64 * (i + 1), :, :])
                for o in range(4):
                    rep_done.add((tensor, s, o))
            elif olist == "rest":
                for i in range(2):
                    nc.sync.dma_start(dstP[0:64, 1:4, i, :],
                                      stage[64 * i:64 * (i + 1), 1:4, :])
                for o in (1, 2, 3):
                    rep_done.add((tensor, s, o))
            else:
                for o in olist:
                    for i in range(2):
                        nc.sync.dma_start(dstP[0:64, o, i, :],
                                          stage[64 * i:64 * (i + 1), o, :])
                    rep_done.add((tensor, s, o))
            yield 1.0

        def gen_v(s, tt):
            """v projection group: t chunk 4s+tt, all 8 heads."""
            xs = xs_tiles[s]
            pv = dense_ps.tile([128, 512], F32, tag="dp", name="pv")
            for c in range(4):
                nc.tensor.matmul(pv[:, :], xs[:, c, 128 * tt:128 * (tt + 1)],
                                 wv[:, c, :], start=(c == 0), stop=False)
            yield
            for c in range(4, CCH):
                nc.tensor.matmul(pv[:, :], xs[:, c, 128 * tt:128 * (tt + 1)],
                                 wv[:, c, :], start=False, stop=(c == CCH - 1))
            nc.vector.tensor_copy(
                vs[s][:, tt, :, 0:D],
                pv[:, :].rearrange("p (h d) -> p h d", h=HPC))
            v_done.add((s, tt))
            yield

        def gen_po(tt, pool=None, cp="dve"):
            """output projection for t chunk tt (both 512-col halves)."""
            pool = pool or dense_ps
            tail = pool is not dense_ps
            while any((tt // 4, h) not in fin_done for h in range(HPC // 2)):
                yield QUANTUM   # this tile's avT not fully written yet
            ob = ob_pool.tile([128, C], BF16, tag="ob", name="ob")
            for o2 in range(2):
                po = pool.tile([128, 512], F32, tag="dp" if not tail else "s",
                               name="po")
                tl = tt % 4
                for c4 in range(GC // 128):
                    nc.tensor.matmul(po[:, :],
                                     avTs[tt // 4][c4][:, 128 * tl:128 * (tl + 1)],
                                     wp[:, c4, 512 * o2:512 * (o2 + 1)],
                                     start=(c4 == 0), stop=(c4 == GC // 128 - 1))
                if (tail and o2 == 0) or cp == "act":
                    nc.scalar.activation(ob[:, 512 * o2:512 * (o2 + 1)], po[:, :], AF.Copy)
                else:
                    nc.vector.tensor_copy(ob[:, 512 * o2:512 * (o2 + 1)], po[:, :])
                if o2 == 1:
                    nc.sync.dma_start(out_d[128 * tt:128 * (tt + 1), :], ob[:, :])
                yield

        # ---- weave machinery ----
        queue = []          # pending dense generators (FIFO)
        debt = [0.0]

        def weave(extra_ns, pop=True):
            debt[0] += extra_ns
            while pop and debt[0] >= QUANTUM * 0.5 and queue:
                g = queue[0]
                try:
                    cost = next(g)
                    debt[0] -= cost if cost else QUANTUM
                except StopIteration:
                    queue.pop(0)

        def flush():
            while queue:
                g = queue.pop(0)
                for _ in g:
                    pass
            debt[0] = 0.0

        def drain_until(pred):
            while not pred() and queue:
                g = queue[0]
                try:
                    next(g)
                except StopIteration:
                    queue.pop(0)

        def enqueue_qk_o(s, o, qstage, kstage):
            queue.append(gen_qk(s, o, wq, qstage))
            queue.append(gen_qk(s, o, wk, kstage))
            queue.append(gen_repack(s, "q", qstage, qPs[s], olist=[o]))
            queue.append(gen_repack(s, "k", kstage, kPs[s], olist=[o]))

        def enqueue_po(tile_j):
            for tt in range(4 * tile_j, 4 * tile_j + 4):
                queue.append(gen_po(tt))

        # last tile-group's outproj accumulates per-c4 partials in SBUF so
        # the c4<=2 matmuls weave into the ACT-bound A(3,*) units; only the
        # c4=3 slice (gated by the last finalize) remains in the tail
        po_sb = [persist.tile([128, C], F32, name=f"posb{t}") for t in range(4)]

        def gen_po_partial(tt, c4):
            tl = tt % 4
            while (3, c4) not in fin_done:
                yield QUANTUM   # avT[3][c4] not written yet; back off
            for o2 in range(2):
                po = dense_ps.tile([128, 512], F32, tag="dp", name="pp")
                nc.tensor.matmul(po[:, :],
                                 avTs[3][c4][:, 128 * tl:128 * (tl + 1)],
                                 wp[:, c4, 512 * o2:512 * (o2 + 1)],
                                 start=True, stop=True)
                dst = po_sb[tt - 12][:, 512 * o2:512 * (o2 + 1)]
                if c4 == 0:
                    nc.vector.tensor_copy(dst, po[:, :])
                else:
                    nc.vector.tensor_tensor(dst, dst, po[:, :], op=ADD)
                yield 213.0

        # ---- attention ----
        # Deferred-work FIFO: each unit's AV chunks (j=0 only) and its
        # last-AV + finalize run one unit later, popped right after the next
        # unit's score chunks. This (a) pipelines unit boundaries (the next
        # exp is always queued before the previous unit's AV tail), and
        # (b) lets the j=0 exps run back-to-back during the startup DMA
        # window before v(0) even exists.
        deferred = []

        def make_fin(j, hp, av0, av1):
            avs = (av0, av1)

            def fin():
                # per-q-partition denominators -> reciprocal (DVE), then
                # broadcast-multiply along d (Pool), PE transpose to [d, q],
                # one Pool copy into avT.
                r = r_pool.tile([128, 2, 4], F32, tag="r", name="r")
                avn = avn_pool.tile([128, 4, 2, 64], F32, tag="avn", name="avn")
                for par in range(2):
                    nc.vector.reciprocal(r[:, par, :], avs[par][:, :, 64])
                for par in range(2):
                    nc.vector.tensor_tensor(
                        avn[:, :, par, :], avs[par][:, :, 0:64],
                        r[:, par, :].unsqueeze(2).broadcast_to([128, 4, 64]),
                        op=MULT)
                av3 = dense_ps.tile([128, 512], F32, tag="dp", name="av3")
                a3 = av3.rearrange("p (c q) -> p c q", c=4)
                for c in range(4):
                    nc.tensor.matmul(a3[:, c, :], avn[:, c, :, :], ident[:, :],
                                     is_transpose=True)
                nc.vector.tensor_copy(avTs[j][hp][:, :], av3[:, :])
                fin_done.add((j, hp))
            return fin

        def emit_att(j, hp):
            nkc = 4 * (j + 1)
            av0 = ps_av.tile([128, 4, 65], F32, tag="av0", name="av0")
            av1 = ps_av.tile([128, 4, 65], F32, tag="av1", name="av1")
            avs = (av0, av1)
            ptiles = {}

            def s_chunk(i):
                roff = max(0, 128 * i - 512 * j)
                sps = ps_s.tile([128, 1024], F32, tag="s", name="sps")
                for par in range(2):
                    pb = 32 * par
                    nc.tensor.matmul(
                        sps[:, 512 * par + roff:512 * (par + 1)],
                        kPs[i // 4][pb:pb + 32, hp, :, 128 * (i % 4):128 * (i % 4 + 1)],
                        qPs[j][pb:pb + 32, hp, :, roff:512],
                        start=True, stop=True, perf_mode=PM.DoubleRow)
                ptile = pt_pool.tile([128, 1024], BF16, tag="pt", name="pt")
                # one exp for both pars: strided AP over the two [roff:512] ranges
                nc.scalar.activation(
                    ptile.rearrange("p (a b) -> p a b", a=2)[:, :, roff:512],
                    sps.rearrange("p (a b) -> p a b", a=2)[:, :, roff:512],
                    AF.Exp, scale=0.125)
                if 128 * i - 512 * j >= 0:
                    for par in range(2):
                        # causal mask: zero p above the diagonal (Pool, SBUF-only)
                        nc.gpsimd.affine_select(
                            out=ptile[:, 512 * par + roff:512 * par + roff + 128],
                            in_=ptile[:, 512 * par + roff:512 * par + roff + 128],
                            compare_op=mybir.AluOpType.is_ge, fill=0.0, base=0,
                            pattern=[[1, 128]], channel_multiplier=-1,
                        )
                return (ptile, roff)

            def make_av(c, par):
                # one q-chunk's full accumulation: a single consecutive
                # start..stop group -- real PSUM banks only support one open
                # accumulation group at a time (interleaved groups corrupt on
                # HW even though CoreSim accepts them)
                def go():
                    last = 4 * j + c
                    for i in range(0, last + 1):
                        if (i // 4, i % 4) not in v_done:
                            drain_until(lambda s=i // 4, t=i % 4:
                                        (s, t) in v_done)
                        nc.tensor.matmul(
                            avs[par][:, c, 0:65],
                            ptiles[i][:, 512 * par + 128 * c:
                                      512 * par + 128 * (c + 1)],
                            vs[i // 4][:, i % 4, 2 * hp + par, :],
                            start=(i == 0), stop=(i == last))
                return go

            def deficit(i):
                roff = max(0, 128 * i - 512 * j)
                n = 512 - roff
                act = ACT_OH + ACT_EL * 2 * n
                pe = (0.5 * n + (n // 128) * 2 * 65) * PE_ROW
                return act - pe

            def pop_deferred():
                n = 1 + (len(deferred) > 6) + (len(deferred) > 10) \
                    + (len(deferred) > 14)
                for _ in range(n):
                    if deferred:
                        deferred.pop(0)()

            side = []
            for i in range(nkc):
                ptiles[i] = s_chunk(i)[0]
                pop_deferred()
                if i > 0:
                    weave(deficit(i - 1), pop=(128 * (i - 1) - 512 * j) < 256)
                if i >= 4 * j:
                    # chunk c = i-4j's inputs are now all emitted
                    tgt = side if j == 0 else deferred
                    tgt.append(make_av(i - 4 * j, 0))
                    tgt.append(make_av(i - 4 * j, 1))
            weave(deficit(nkc - 1), pop=False)
            deferred.extend(side)
            deferred.append(make_fin(j, hp, av0, av1))

        # ---- main schedule ----
        # Startup: only q/k group 0 + its repack run inline, so ATT(0,0)'s
        # first exp fires ~3us in. Everything else sits in one global FIFO
        # ordered by need-by time; the weave pulls it into attention-unit
        # deficit gaps and the rep/v guards force-drain punctually when the
        # schedule runs tight. PO group j is queued after fins(j) exist.
        def run_now(g):
            for _ in g:
                pass

        stages = {}
        for s in range(NSTRIP):
            stages[s] = (
                st_pool.tile([128, 4, 512], FP8, tag="st8", name=f"qs{s}"),
                st_pool.tile([128, 4, 512], FP8, tag="st8", name=f"ks{s}"))
        q0stage, k0stage = stages[0]
        run_now(gen_qk(0, 0, wq, q0stage))
        run_now(gen_qk(0, 0, wk, k0stage))
        run_now(gen_repack(0, "q", q0stage, qPs[0], olist=[0]))
        run_now(gen_repack(0, "k", k0stage, kPs[0], olist=[0]))
        load_xs0_wv()
        load_qk_rest()
        for s in range(NSTRIP):
            qstage, kstage = stages[s]
            if s == 0:
                # strip 0: all q/k groups first -- the j=0 AVs are deferred a
                # unit anyway, and v(0) would stall the PE on the bulk loads
                for o in range(1, 4):
                    enqueue_qk_o(0, o, qstage, kstage)
                for tt in range(4):
                    queue.append(gen_v(0, tt))
                continue
            enqueue_qk_o(s, 0, qstage, kstage)
            queue.append(gen_v(s, 0))
            queue.append(gen_v(s, 1))
            for o in range(1, 4):
                queue.append(gen_qk(s, o, wq, qstage))
                queue.append(gen_qk(s, o, wk, kstage))
            queue.append(gen_repack(s, "q", qstage, qPs[s], olist="rest"))
            queue.append(gen_repack(s, "k", kstage, kPs[s], olist="rest"))
            queue.append(gen_v(s, 2))
            queue.append(gen_v(s, 3))
            if s == 2:
                enqueue_po(0)
                enqueue_po(1)

        def load_wp():
            nc.gpsimd.dma_start(wp[:, :, :],
                                wp_d[:, :].rearrange("(c p) o -> p c o", p=128))

        def enqueue_po_partials(c4):
            for tt in range(12, 16):
                queue.append(gen_po_partial(tt, c4))

        pre_jobs = {
            (0, 0): [lambda: load_strip(1)],
            (1, 0): [lambda: load_strip(2), load_wp],
            (2, 0): [lambda: load_strip(3)],
            # partial-c4 jobs are enqueued one unit after fin(3,c4) is
            # guaranteed emitted (it pops at A(3,c4+1)'s second chunk);
            # POg2 queued late so it weaves into A(3,2)+ and keeps the PE
            # warm right before the tail
            (3, 1): [lambda: enqueue_po_partials(0)],
            (3, 2): [lambda: enqueue_po(2), lambda: enqueue_po_partials(1)],
            (3, 3): [lambda: enqueue_po_partials(2)],
        }
        for j in range(NJ):
            for hp in range(HPC // 2):
                for f in pre_jobs.get((j, hp), []):
                    f()
                drain_until(lambda: ("q", j, hp) in rep_done
                            and ("k", j, hp) in rep_done)
                emit_att(j, hp)
        while deferred:
            deferred.pop(0)()
        flush()
        # tail: only the c4=3 slice of tt 12..15 (gated by fin(3,3)) plus the
        # bf16 merge with the SBUF partial sums, pipelined across the psum
        # rings; a couple of zero-matmuls bridge the finalize latency
        for _ in range(3):
            nc.tensor.matmul(dense_ps.tile([128, 512], F32, tag="dp", name="wz"),
                             warm[:, :], avTs[0][0][:, :], start=True, stop=True)
        obs = {}
        for tt in range(12, 16):
            tl = tt % 4
            obs[tt] = ob_pool.tile([128, C], BF16, tag="ob", name="ob")
            for o2 in range(2):
                po = ps_s.tile([128, 512], F32, tag="s", name="po") \
                    if (2 * tt + o2) % 4 < 2 else \
                    dense_ps.tile([128, 512], F32, tag="dp", name="po")
                nc.tensor.matmul(po[:, :],
                                 avTs[3][3][:, 128 * tl:128 * (tl + 1)],
                                 wp[:, 3, 512 * o2:512 * (o2 + 1)],
                                 start=True, stop=True)
                dst = obs[tt][:, 512 * o2:512 * (o2 + 1)]
                nc.vector.tensor_tensor(
                    dst, po_sb[tt - 12][:, 512 * o2:512 * (o2 + 1)], po[:, :],
                    op=ADD)
                if o2 == 1:
                    nc.sync.dma_start(out_d[128 * tt:128 * (tt + 1), :],
                                      obs[tt][:, :])
    _split_excess_waits(nc)
    return nc


def _get_program():
    global _PROGRAM
    if _PROGRAM is None:
        _PROGRAM = _build_program()
    return _PROGRAM


def _qk_perm():
    """Column perm for Wq/Wk: channel c = 64*par + 2*l + i (within an o-group
    of 128) is placed at position p = 64*i + 32*par + l, so the on-device
    fp8 stage can be repacked into DoubleRow layout with two plain
    partition-block DMAs."""
    perm = np.empty(128, dtype=np.int64)
    for p in range(128):
        i, rem = divmod(p, 64)
        par, l = divmod(rem, 32)
        perm[p] = 64 * par + 2 * l + i
    full = np.concatenate([128 * o + perm for o in range(4)])
    return full


def _make_in_maps(x, Wk, Wq, Wv, Wp):
    import ml_dtypes
    x = np.asarray(x, dtype=np.float32)
    Wk = np.asarray(Wk, dtype=np.float32)
    Wq = np.asarray(Wq, dtype=np.float32)
    Wv = np.asarray(Wv, dtype=np.float32)
    Wp = np.asarray(Wp, dtype=np.float32)
    perm = _qk_perm()
    ident = np.eye(128, dtype=np.float32)
    in_maps = []
    for core in range(8):
        b, g = core // GROUPS, core % GROUPS
        rows = slice(GC * g, GC * (g + 1))
        wqT = np.ascontiguousarray(Wq[rows, :].T)[:, perm]
        wkT = np.ascontiguousarray(Wk[rows, :].T)[:, perm]
        in_maps.append({
            "xT": np.ascontiguousarray(x[b].T).astype(ml_dtypes.bfloat16),
            "xT8": np.ascontiguousarray(x[b].T).astype(ml_dtypes.float8_e4m3),
            "wqT": np.ascontiguousarray(wqT).astype(ml_dtypes.float8_e4m3),
            "wkT": np.ascontiguousarray(wkT).astype(ml_dtypes.float8_e4m3),
            "wvT": np.ascontiguousarray(Wv[rows, :].T).astype(ml_dtypes.bfloat16),
            "wpT": np.ascontiguousarray(Wp[:, rows].T).astype(ml_dtypes.bfloat16),
            "ident": ident,
        })
    return in_maps


def run(x, Wk, Wq, Wv, Wp, bp, trace=False, **spmd_kwargs):
    nc = _get_program()
    in_maps = _make_in_maps(x, Wk, Wq, Wv, Wp)
    res = run_bass_kernel_spmd(nc, in_maps, list(range(8)), trace=trace, **spmd_kwargs)
    bp = np.asarray(bp, dtype=np.float32)
    out = np.empty((B, T, C), dtype=np.float32)
    for b in range(B):
        out[b] = (res.results[GROUPS * b]["outp"].astype(np.float32)
                  + res.results[GROUPS * b + 1]["outp"].astype(np.float32) + bp)
    return out, res


def kernel(x, Wk, Wq, Wv, Wp, bp):
    out, _ = run(x, Wk, Wq, Wv, Wp, bp)
    return out
